# revision 14
# baseline (speedup 1.0000x reference)
"""CNF GNN message-passing kernel for trn2 (8 NeuronCores, SPMD).

Sharding: edges partitioned by DESTINATION (clause shard for l2c, literal
shard for c2l) so each core's local segment-sum is the full sum - no
all-reduce of node features. The linear-layer outputs (Wh tables) are
all-gathered (fp16, split in halves for overlap) so every core can gather
rows for its edges.

Aggregation per direction processes dest BLOCKS of 256 rows (one [P, 256]
psum per block). Edge chunks (128 edges) are pure in (block, source
segment); gathers use InstDMAGatherAnt with int16 in-segment indices,
batched up to 8 chunks (1024 rows) per instruction to amortize the ~1us
SWDGE descriptor-generation overhead (the old per-chunk indirect DMA was
Pool-engine-bound at ~5.3 ms). Blocks are processed in quads so one
gather instruction can span 4 blocks' chunks of the same segment.

Per iteration:
  stage A: Wh_l2c shard = tied_lembs @ Wl  (PE)          -> AllGather #1
  stage C: per block: gather Wh_l2c rows, one-hot (rel==iota)*rc on DVE,
           psum[f, c] += g.T @ oh; relu -> cembs_T [P, 256]
  stage D (fused): Wh_c2l = cembs @ Wc + clause_feat x wcf -> AllGather #2
  stage E: same aggregation into lembs_T; pair-swap for literal tying;
           next stage A matmuls (or final transpose to output).
"""
import numpy as np
from dataclasses import dataclass, field

import concourse.bass as bass
import concourse.bass_isa as bass_isa
import concourse.mybir as mybir
import concourse.tile as tile
from concourse import library_config

F32 = mybir.dt.float32
F16 = mybir.dt.float16
I32 = mybir.dt.int32
I16 = mybir.dt.int16
P = 128
BW = 256            # dest block width (2 ranges of 128)
QUAD = 4            # blocks per processing group
GB = 8              # chunks per dma_gather instruction (8*128 = 1024 idx)
SEG1 = 25088        # whl table segment rows (4 segs over 8*12544)
SEG2 = 28672        # whc table segment rows (7 segs over 8*25088)


# ---------------------------------------------------------------- host prep

@dataclass
class DirData:
    """Per-direction schedule (shared across cores) + per-core arrays."""
    nblk: int
    nseg: int
    NC: int                      # total chunk columns
    sched: list                  # [quad][seg] -> list of (c0, nch)
    chunk_slot: list             # column -> block slot
    slot_nchunks: list           # slot -> total chunks (0 => memset)
    # per-core packed arrays are stored in CoreData.in_map


@dataclass
class CoreData:
    in_map: dict
    perm1: np.ndarray = None
    perm2: np.ndarray = None


@dataclass
class Problem:
    L: int
    C: int
    E: int
    VLAB: int
    D: int
    ITERS: int
    ncores: int
    lsh_true: int
    csh_true: int
    LSH: int
    CSH: int
    NR1: int
    NR2: int
    d1: DirData = None
    d2: DirData = None
    cores: list = field(default_factory=list)


def _pack_idx16(vals_by_col):
    """vals_by_col: [NC][128] int array -> replicated [128, 8*NC] int16."""
    NC = len(vals_by_col)
    arr = np.zeros((16, 8 * NC), np.int16)
    for c, vals in enumerate(vals_by_col):
        v = np.asarray(vals, np.int16).reshape(8, 16)   # i -> [i//16, i%16]
        arr[:, c * 8:(c + 1) * 8] = v.T
    return np.ascontiguousarray(np.tile(arr, (8, 1)))


def _bucket_dir(dst_loc, src_grow, rc_edge, nblk, nseg, segrows):
    """Bucket a core's edges by (block slot, src segment). Returns
    dict[(blk, seg)] -> (idx_in_seg, rel, rc) arrays and per-blk counts."""
    blk = dst_loc // BW
    rel = (dst_loc % BW).astype(np.float32)
    seg = src_grow // segrows
    idxv = (src_grow - seg * segrows).astype(np.int32)
    buckets = {}
    order = np.lexsort((dst_loc, seg, blk))
    blk_s, seg_s = blk[order], seg[order]
    idx_s, rel_s, rc_s = idxv[order], rel[order], rc_edge[order]
    b_edges = np.bincount(blk, minlength=nblk)
    # split on (blk, seg) boundaries
    key = blk_s * nseg + seg_s
    cut = np.flatnonzero(np.diff(key)) + 1
    starts = np.concatenate([[0], cut])
    ends = np.concatenate([cut, [len(key)]])
    for a, b in zip(starts, ends):
        if a == b:
            continue
        buckets[(int(blk_s[a]), int(seg_s[a]))] = (
            idx_s[a:b], rel_s[a:b], rc_s[a:b])
    return buckets, b_edges


def _build_direction(per_core_buckets, per_core_perm, nblk, nseg):
    """Cross-core aligned chunk schedule + per-core packed arrays.

    per_core_buckets[k]: dict[(real_blk, seg)] -> (idx, rel, rc)
    per_core_perm[k]: slot -> real block
    """
    ncores = len(per_core_buckets)
    common = np.zeros((nblk, nseg), np.int64)
    for k in range(ncores):
        perm = per_core_perm[k]
        for (rb, m), (gi, _, _) in per_core_buckets[k].items():
            s = int(np.flatnonzero(perm == rb)[0])
            common[s, m] = max(common[s, m], (len(gi) + P - 1) // P)
    # column layout: quads, then segments, then slots-in-quad
    sched, chunk_slot = [], []
    col = 0
    nquads = (nblk + QUAD - 1) // QUAD
    for q in range(nquads):
        slots_q = list(range(q * QUAD, min((q + 1) * QUAD, nblk)))
        per_m = []
        for m in range(nseg):
            cols_m = []
            for s in slots_q:
                cols_m.extend([s] * int(common[s, m]))
            batches = []
            i0 = 0
            while i0 < len(cols_m):
                nch = min(GB, len(cols_m) - i0)
                batches.append((col + i0, nch))
                i0 += nch
            chunk_slot.extend(cols_m)
            col += len(cols_m)
            per_m.append(batches)
        sched.append(per_m)
    NC = col
    slot_nchunks = [int(common[s].sum()) for s in range(nblk)]

    # per-core packed arrays in the common layout
    packed = []
    for k in range(ncores):
        perm = per_core_perm[k]
        idx_cols, rel_cols, rc_cols = [], [], []
        for q in range(nquads):
            slots_q = list(range(q * QUAD, min((q + 1) * QUAD, nblk)))
            for m in range(nseg):
                for s in slots_q:
                    nch = int(common[s, m])
                    if nch == 0:
                        continue
                    gi, rl, rcv = per_core_buckets[k].get(
                        (int(perm[s]), m), (np.zeros(0, np.int32),
                                            np.zeros(0, np.float32),
                                            np.zeros(0, np.float32)))
                    pad = nch * P - len(gi)
                    gi = np.concatenate([gi, np.zeros(pad, np.int32)])
                    rl = np.concatenate([rl, np.full(pad, -1.0, np.float32)])
                    rcv = np.concatenate([rcv, np.zeros(pad, np.float32)])
                    for j in range(nch):
                        idx_cols.append(gi[j * P:(j + 1) * P])
                        rel_cols.append(rl[j * P:(j + 1) * P])
                        rc_cols.append(rcv[j * P:(j + 1) * P])
        assert len(idx_cols) == NC
        idx16 = _pack_idx16(idx_cols)
        rel = np.stack(rel_cols, axis=1).astype(np.float32)
        rc = np.stack(rc_cols, axis=1).astype(np.float32)
        packed.append((idx16, np.ascontiguousarray(rel),
                       np.ascontiguousarray(rc)))
    dd = DirData(nblk=nblk, nseg=nseg, NC=NC, sched=sched,
                 chunk_slot=chunk_slot, slot_nchunks=slot_nchunks)
    return dd, packed


def prepare(inputs, L, C, E, VLAB=8, D=128, ITERS=3, ncores=8):
    lit_feat = np.asarray(inputs["lit_feat"], np.float32)
    clause_feat = np.asarray(inputs["clause_feat"], np.float32).reshape(-1)
    e_lit = np.asarray(inputs["edge_lit"], np.int64)
    e_cls = np.asarray(inputs["edge_clause"], np.int64)

    assert L % ncores == 0 and C % ncores == 0
    lsh_true, csh_true = L // ncores, C // ncores
    LSH = ((lsh_true + P - 1) // P) * P
    CSH = ((csh_true + P - 1) // P) * P
    NR1, NR2 = CSH // P, LSH // P
    nblk1, nblk2 = CSH // BW, LSH // BW
    LHALF, CHALF = LSH // 2, CSH // 2
    nseg1 = (ncores * LSH + SEG1 - 1) // SEG1
    nseg2 = (ncores * CSH + SEG2 - 1) // SEG2
    assert SEG1 * nseg1 >= ncores * LSH and SEG2 * nseg2 >= ncores * CSH

    cnt_c = np.bincount(e_cls, minlength=C).astype(np.float32)
    cnt_l = np.bincount(e_lit, minlength=L).astype(np.float32)
    rc_c = (1.0 / np.maximum(cnt_c, 1.0)).astype(np.float32)
    rc_l = (1.0 / np.maximum(cnt_l, 1.0)).astype(np.float32)

    W0 = np.asarray(inputs["W_l2c0"], np.float32)
    b0 = np.asarray(inputs["b_l2c0"], np.float32)
    Wl = np.asarray(inputs["W_l2c"], np.float32)
    bl = np.asarray(inputs["b_l2c"], np.float32)
    Wc = np.asarray(inputs["W_c2l"], np.float32)
    bc = np.asarray(inputs["b_c2l"], np.float32)
    have_bias = bool(np.any(b0) or np.any(bl) or np.any(bc))
    Wc_main = np.concatenate([Wc[i, :D, :] for i in range(ITERS)], axis=0)
    wcf = np.stack([Wc[i, D, :] for i in range(ITERS)], axis=0)
    Wl_pack = Wl.reshape((ITERS - 1) * 2 * D, D) if ITERS > 1 else np.zeros((1, D), np.float32)
    bias_pack = np.concatenate(
        [b0[None, :], bl if ITERS > 1 else np.zeros((0, D), np.float32), bc],
        axis=0)

    # per-core block permutations (sorted by block size desc)
    perms1, perms2, inv1, inv2 = [], [], [], []
    sel1s, sel2s = [], []
    for k in range(ncores):
        sel1 = (e_cls >= k * csh_true) & (e_cls < (k + 1) * csh_true)
        loc1 = e_cls[sel1] - k * csh_true
        sizes1 = np.bincount(loc1 // BW, minlength=nblk1)
        p1 = np.argsort(-sizes1, kind="stable")
        perms1.append(p1)
        inv1.append(np.argsort(p1))
        sel1s.append(sel1)
        sel2 = (e_lit >= k * lsh_true) & (e_lit < (k + 1) * lsh_true)
        loc2 = e_lit[sel2] - k * lsh_true
        sizes2 = np.bincount(loc2 // BW, minlength=nblk2)
        p2 = np.argsort(-sizes2, kind="stable")
        perms2.append(p2)
        inv2.append(np.argsort(p2))
        sel2s.append(sel2)

    inv1_all = np.stack(inv1)   # [ncores, nblk1]
    inv2_all = np.stack(inv2)

    def glit_slot(l):
        k, loc = l // lsh_true, l % lsh_true
        Bl, w = loc // BW, loc % BW
        S = inv2_all[k, Bl]
        sr = S * BW + w
        hi = (sr >= LHALF).astype(np.int64)
        return hi * (ncores * LHALF) + k * LHALF + (sr - hi * LHALF)

    def gcls_slot(c):
        k, loc = c // csh_true, c % csh_true
        Bl, w = loc // BW, loc % BW
        S = inv1_all[k, Bl]
        sr = S * BW + w
        hi = (sr >= CHALF).astype(np.int64)
        return hi * (ncores * CHALF) + k * CHALF + (sr - hi * CHALF)

    # bucket edges per core per direction (dest rows in SLOT space)
    bks1, bks2 = [], []
    for k in range(ncores):
        sel1, sel2 = sel1s[k], sel2s[k]
        loc1 = e_cls[sel1] - k * csh_true
        slot_loc1 = inv1_all[k, loc1 // BW] * BW + loc1 % BW
        b1, _ = _bucket_dir(slot_loc1, glit_slot(e_lit[sel1]),
                            rc_c[e_cls[sel1]], nblk1, nseg1, SEG1)
        bks1.append(b1)
        loc2 = e_lit[sel2] - k * lsh_true
        slot_loc2 = inv2_all[k, loc2 // BW] * BW + loc2 % BW
        b2, _ = _bucket_dir(slot_loc2, gcls_slot(e_cls[sel2]),
                            rc_l[e_lit[sel2]], nblk2, nseg2, SEG2)
        bks2.append(b2)

    # schedules reference SLOT ids; buckets keyed by (real?, seg).
    # _build_direction maps slot->real via perm; but buckets above are
    # already keyed by slot-projected... keep keys as (slot, seg):
    # _bucket_dir bucketed by slot_loc//BW == slot directly, so pass
    # identity perms.
    ident1 = [np.arange(nblk1) for _ in range(ncores)]
    ident2 = [np.arange(nblk2) for _ in range(ncores)]
    d1, packed1 = _build_direction(bks1, ident1, nblk1, nseg1)
    d2, packed2 = _build_direction(bks2, ident2, nblk2, nseg2)

    cores = []
    for k in range(ncores):
        idx16_1, rel1, rc1 = packed1[k]
        idx16_2, rel2, rc2 = packed2[k]
        litT0 = np.zeros((VLAB, LSH), np.float32)
        litT0[:, :lsh_true] = lit_feat[k * lsh_true:(k + 1) * lsh_true].T
        litT = np.ascontiguousarray(
            litT0.reshape(VLAB, nblk2, BW)[:, perms2[k], :].reshape(VLAB, LSH))
        cfk_flat = np.zeros(CSH, np.float32)
        cfk_flat[:csh_true] = clause_feat[k * csh_true:(k + 1) * csh_true]
        cfk = np.ascontiguousarray(
            cfk_flat.reshape(nblk1, BW)[perms1[k]].reshape(NR1, P).T)
        iota256 = np.tile(np.arange(BW, dtype=np.float16), (P, 1))
        ident = np.eye(P, dtype=np.float32)
        in_map = dict(
            litT=litT, cf=cfk,
            idx1=idx16_1, rel1=rel1, rc1=rc1,
            idx2=idx16_2, rel2=rel2, rc2=rc2,
            W0=W0, Wc_main=Wc_main, wcf=wcf, Wl_pack=Wl_pack,
            bias_pack=bias_pack, iota256=iota256, ident=ident,
        )
        cores.append(CoreData(in_map, perms1[k], perms2[k]))

    p = Problem(L, C, E, VLAB, D, ITERS, ncores, lsh_true, csh_true,
                LSH, CSH, NR1, NR2, d1, d2, cores)
    return p, have_bias


# ---------------------------------------------------------------- kernel

def build(prob: Problem, have_bias=False, gather_bufs=6,
          oneh_bufs=6, work_bufs=4, psA_bufs=1, psD_bufs=1, psT_bufs=2,
          table_dtype=F16, dma_scratch=16384):
    d1, d2 = prob.d1, prob.d2
    NC1, NC2 = d1.NC, d2.NC
    L, C, D, VLAB, ITERS = prob.L, prob.C, prob.D, prob.VLAB, prob.ITERS
    LSH, CSH, NR1, NR2 = prob.LSH, prob.CSH, prob.NR1, prob.NR2
    ncores = prob.ncores
    Lfull, Cfull = ncores * LSH, ncores * CSH
    TD = table_dtype

    nc = bass.Bass(dynamic_dma_scratch_size=dma_scratch)
    litT = nc.dram_tensor("litT", [VLAB, LSH], F32, kind="ExternalInput")
    cf = nc.dram_tensor("cf", [P, NR1], F32, kind="ExternalInput")
    idx1 = nc.dram_tensor("idx1", [P, 8 * NC1], I16, kind="ExternalInput")
    rel1 = nc.dram_tensor("rel1", [P, NC1], F32, kind="ExternalInput")
    rc1 = nc.dram_tensor("rc1", [P, NC1], F32, kind="ExternalInput")
    idx2 = nc.dram_tensor("idx2", [P, 8 * NC2], I16, kind="ExternalInput")
    rel2 = nc.dram_tensor("rel2", [P, NC2], F32, kind="ExternalInput")
    rc2 = nc.dram_tensor("rc2", [P, NC2], F32, kind="ExternalInput")
    W0 = nc.dram_tensor("W0", [VLAB, D], F32, kind="ExternalInput")
    Wc_main = nc.dram_tensor("Wc_main", [ITERS * D, D], F32, kind="ExternalInput")
    wcf = nc.dram_tensor("wcf", [ITERS, D], F32, kind="ExternalInput")
    Wl_pack = nc.dram_tensor("Wl_pack", [max((ITERS - 1) * 2 * D, 1), D], F32, kind="ExternalInput")
    bias_pack = nc.dram_tensor("bias_pack", [2 * ITERS, D], F32, kind="ExternalInput")
    iota256 = nc.dram_tensor("iota256", [P, BW], F16, kind="ExternalInput")
    identin = nc.dram_tensor("ident", [P, P], F32, kind="ExternalInput")
    out = nc.dram_tensor("out", [LSH, 2 * D], F32, kind="ExternalOutput")

    shared = "Shared" if ncores > 4 else "Local"
    whl_bounce_a = nc.dram_tensor("whl_bounce_a", [LSH // 2, D], TD)
    whl_bounce_b = nc.dram_tensor("whl_bounce_b", [LSH // 2, D], TD)
    whl_full = nc.dram_tensor("whl_full", [Lfull, D], TD, addr_space=shared)
    whc_bounce_a = nc.dram_tensor("whc_bounce_a", [CSH // 2, D], TD)
    whc_bounce_b = nc.dram_tensor("whc_bounce_b", [CSH // 2, D], TD)
    whc_full = nc.dram_tensor("whc_full", [Cfull, D], TD, addr_space=shared)

    rg = [list(range(ncores))]
    LHALF, CHALF = LSH // 2, CSH // 2

    with tile.TileContext(nc) as tc:
        with (
            tc.tile_pool(name="const", bufs=1) as constp,
            tc.tile_pool(name="meta", bufs=1) as metap,
            tc.tile_pool(name="gath", bufs=gather_bufs) as gathp,
            tc.tile_pool(name="oneh", bufs=oneh_bufs) as onehp,
            tc.tile_pool(name="work", bufs=work_bufs) as workp,
            tc.tile_pool(name="outw", bufs=4) as outwp,
            tc.tile_pool(name="lit", bufs=3) as litp,
            tc.tile_pool(name="psA", bufs=psA_bufs, space="PSUM") as psA,
            tc.tile_pool(name="psD", bufs=psD_bufs, space="PSUM") as psD,
            tc.tile_pool(name="psT", bufs=psT_bufs, space="PSUM") as psT,
        ):
            # load the Q7 library for InstDMAGatherAnt before any gather
            nc.gpsimd.load_library(library_config.mlp)

            # one Pool register per distinct num_idxs value (to_reg leaks a
            # fresh register per call otherwise)
            nidx_regs = {}

            def nidx_reg(v):
                if v not in nidx_regs:
                    nidx_regs[v] = nc.gpsimd.to_reg(v)
                return nidx_regs[v]

            # ---- constants
            iota_f = constp.tile([P, BW], F16)
            nc.sync.dma_start(out=iota_f[:], in_=iota256[:, :])
            ident = constp.tile([P, P], F32)
            nc.sync.dma_start(out=ident[:], in_=identin[:, :])

            W0_sb = constp.tile([VLAB, D], F32)
            nc.sync.dma_start(out=W0_sb[:], in_=W0[:, :])
            Wc_sb = constp.tile([P, ITERS * D], F32)
            for i in range(ITERS):
                nc.sync.dma_start(out=Wc_sb[:, i * D:(i + 1) * D],
                                  in_=Wc_main[i * P:(i + 1) * P, :])
            wcf_sb = constp.tile([1, ITERS * D], F32)
            for i in range(ITERS):
                nc.sync.dma_start(out=wcf_sb[:, i * D:(i + 1) * D], in_=wcf[i:i + 1, :])
            if ITERS > 1:
                Wl_sb = constp.tile([P, (ITERS - 1) * 2 * D], F32)
                for i in range(2 * (ITERS - 1)):
                    nc.sync.dma_start(out=Wl_sb[:, i * D:(i + 1) * D],
                                      in_=Wl_pack[i * P:(i + 1) * P, :])
            bias_sb = constp.tile([1, 2 * ITERS * D], F32)
            for i in range(2 * ITERS):
                nc.sync.dma_start(out=bias_sb[:, i * D:(i + 1) * D],
                                  in_=bias_pack[i:i + 1, :])
            ones_sb = constp.tile([1, P], F32)
            nc.vector.memset(ones_sb[:], 1.0)
            cf_sb = constp.tile([P, NR1], F32)
            nc.sync.dma_start(out=cf_sb[:], in_=cf[:, :])

            # ---- edge metadata
            idx1_sb = metap.tile([P, 8 * NC1], I16)
            rel1_sb = metap.tile([P, NC1], F32)
            rc1_sb = metap.tile([P, NC1], F32)
            idx2_sb = metap.tile([P, 8 * NC2], I16)
            rel2_sb = metap.tile([P, NC2], F32)
            rc2_sb = metap.tile([P, NC2], F32)
            for dst, src in [(idx1_sb, idx1), (rel1_sb, rel1), (rc1_sb, rc1),
                             (idx2_sb, idx2), (rel2_sb, rel2), (rc2_sb, rc2)]:
                nc.sync.dma_start(out=dst[:], in_=src[:, :])

            def agg_direction(dd: DirData, idx_sb, rel_sb, rc_sb, table, segrows):
                """Yield (slot, seg256 tile [P, BW]) per dest block after
                aggregation + relu, in slot order."""
                nquads = (dd.nblk + QUAD - 1) // QUAD
                emitted = [0] * dd.nblk       # chunks consumed per slot
                for q in range(nquads):
                    slots_q = list(range(q * QUAD, min((q + 1) * QUAD, dd.nblk)))
                    ps_of = {}
                    for s in slots_q:
                        if dd.slot_nchunks[s] > 0:
                            ps_agg = psA.tile([P, BW], F32, space="PSUM",
                                              tag=f"agg{s % QUAD}")
                            ps_of[s] = ps_agg
                    for m in range(dd.nseg):
                        for (c0, nch) in dd.sched[q][m]:
                            g = gathp.tile([P, GB, D], TD, tag="g")
                            nc.gpsimd.dma_gather(
                                g[:, 0:nch, :],
                                table[m * segrows:(m + 1) * segrows, :],
                                idx_sb[:, c0 * 8:(c0 + nch) * 8],
                                nch * P, nidx_reg(nch * P), D,
                            )
                            for j in range(nch):
                                col = c0 + j
                                s = dd.chunk_slot[col]
                                oh = onehp.tile([P, BW], F16, tag="oh")
                                nc.vector.tensor_scalar(
                                    out=oh[:], in0=iota_f[:],
                                    scalar1=rel_sb[:, col:col + 1],
                                    scalar2=rc_sb[:, col:col + 1],
                                    op0=mybir.AluOpType.is_equal,
                                    op1=mybir.AluOpType.mult,
                                )
                                nc.tensor.matmul(
                                    out=ps_of[s][:], lhsT=g[:, j, :], rhs=oh[:],
                                    start=(emitted[s] == 0),
                                    stop=(emitted[s] == dd.slot_nchunks[s] - 1))
                                emitted[s] += 1
                    for s in slots_q:
                        seg = workp.tile([P, BW], F32, tag="seg")
                        if dd.slot_nchunks[s] == 0:
                            nc.vector.memset(seg[:], 0.0)
                        else:
                            nc.vector.tensor_scalar_max(seg[:], ps_of[s][:], 0.0)
                        yield s, seg

            def emit_ag(bounces, full, sh, half):
                nc.gpsimd.collective_compute(
                    "AllGather", mybir.AluOpType.bypass,
                    ins=[bounces[half][:].opt()],
                    outs=[full[half * ncores * sh:(half + 1) * ncores * sh, :].opt()],
                    replica_groups=rg)

            def bounce_write(bounces, sh, r, tile_ap):
                half, off = (0, r * P) if r * P < sh else (1, r * P - sh)
                nc.sync.dma_start(out=bounces[half][off:off + P, :], in_=tile_ap)

            whl_bounces = (whl_bounce_a, whl_bounce_b)
            whc_bounces = (whc_bounce_a, whc_bounce_b)

            for it in range(ITERS):
                # ---------- stage A -> whl_bounce (first iteration only)
                if it == 0:
                    for r in range(NR2):
                        lt = litp.tile([VLAB, P], F32, tag="lt")
                        nc.sync.dma_start(out=lt[:], in_=litT[:, r * P:(r + 1) * P])
                        ps = psD.tile([P, D], F32, space="PSUM", tag="whl")
                        nc.tensor.matmul(out=ps[:], lhsT=lt[:], rhs=W0_sb[:],
                                         start=True, stop=not have_bias)
                        if have_bias:
                            nc.tensor.matmul(out=ps[:], lhsT=ones_sb[:],
                                             rhs=bias_sb[:, 0:D],
                                             start=False, stop=True)
                        ot = outwp.tile([P, D], TD, tag="whl_o")
                        nc.scalar.activation(ot[:], ps[:],
                                             mybir.ActivationFunctionType.Copy)
                        bounce_write(whl_bounces, LHALF, r, ot[:])
                        if r == NR2 // 2 - 1:
                            emit_ag(whl_bounces, whl_full, LHALF, 0)
                    emit_ag(whl_bounces, whl_full, LHALF, 1)

                # broadcast wcf[it] across partitions once per iteration
                ps_b = psD.tile([P, D], F32, space="PSUM", tag="whc")
                nc.tensor.matmul(out=ps_b[:], lhsT=ones_sb[:],
                                 rhs=wcf_sb[:, it * D:(it + 1) * D],
                                 start=True, stop=True)
                wcf_bc = workp.tile([P, D], F32, tag="wcfbc")
                nc.scalar.activation(wcf_bc[:], ps_b[:],
                                     mybir.ActivationFunctionType.Copy)

                # ---------- stage C + D: clause blocks
                for s, seg in agg_direction(d1, idx1_sb, rel1_sb, rc1_sb,
                                            whl_full, SEG1):
                    for h in range(2):
                        r = 2 * s + h
                        ps2 = psD.tile([P, D], F32, space="PSUM", tag="whc")
                        nc.tensor.matmul(out=ps2[:],
                                         lhsT=seg[:, h * P:(h + 1) * P],
                                         rhs=Wc_sb[:, it * D:(it + 1) * D],
                                         start=True, stop=not have_bias)
                        if have_bias:
                            nc.tensor.matmul(out=ps2[:], lhsT=ones_sb[:],
                                             rhs=bias_sb[:, (ITERS + it) * D:(ITERS + it + 1) * D],
                                             start=False, stop=True)
                        ot = outwp.tile([P, D], TD, tag="whc_o")
                        nc.vector.scalar_tensor_tensor(
                            out=ot[:], in0=wcf_bc[:], scalar=cf_sb[:, r:r + 1],
                            in1=ps2[:], op0=mybir.AluOpType.mult,
                            op1=mybir.AluOpType.add,
                        )
                        bounce_write(whc_bounces, CHALF, r, ot[:])
                        if r == NR1 // 2 - 1:
                            emit_ag(whc_bounces, whc_full, CHALF, 0)
                emit_ag(whc_bounces, whc_full, CHALF, 1)

                # ---------- stage E: literal blocks
                for s, seg in agg_direction(d2, idx2_sb, rel2_sb, rc2_sb,
                                            whc_full, SEG2):
                    swp = workp.tile([P, BW], F32, tag="swp")
                    nc.vector.tensor_copy(swp[:, 0::2], seg[:, 1::2])
                    nc.vector.tensor_copy(swp[:, 1::2], seg[:, 0::2])
                    for h in range(2):
                        r = 2 * s + h
                        if it < ITERS - 1:
                            ps3 = psD.tile([P, D], F32, space="PSUM", tag="whl")
                            nc.tensor.matmul(out=ps3[:],
                                             lhsT=seg[:, h * P:(h + 1) * P],
                                             rhs=Wl_sb[:, (2 * it) * D:(2 * it + 1) * D],
                                             start=True, stop=False)
                            nc.tensor.matmul(out=ps3[:],
                                             lhsT=swp[:, h * P:(h + 1) * P],
                                             rhs=Wl_sb[:, (2 * it + 1) * D:(2 * it + 2) * D],
                                             start=False, stop=not have_bias)
                            if have_bias:
                                nc.tensor.matmul(out=ps3[:], lhsT=ones_sb[:],
                                                 rhs=bias_sb[:, (1 + it) * D:(2 + it) * D],
                                                 start=False, stop=True)
                            ot = outwp.tile([P, D], TD, tag="whl_o")
                            nc.scalar.activation(ot[:], ps3[:],
                                                 mybir.ActivationFunctionType.Copy)
                            bounce_write(whl_bounces, LHALF, r, ot[:])
                            if r == NR2 // 2 - 1:
                                emit_ag(whl_bounces, whl_full, LHALF, 0)
                            if r == NR2 - 1:
                                emit_ag(whl_bounces, whl_full, LHALF, 1)
                        else:
                            pst = psT.tile([P, P], F32, space="PSUM", tag="tr")
                            nc.tensor.transpose(out=pst[:],
                                                in_=seg[:, h * P:(h + 1) * P],
                                                identity=ident[:])
                            ob = outwp.tile([P, 2 * D], F32, tag="fin")
                            nc.scalar.activation(ob[:, :D], pst[:],
                                                 mybir.ActivationFunctionType.Copy)
                            pst2 = psT.tile([P, P], F32, space="PSUM", tag="tr")
                            nc.tensor.transpose(out=pst2[:],
                                                in_=swp[:, h * P:(h + 1) * P],
                                                identity=ident[:])
                            nc.scalar.activation(ob[:, D:], pst2[:],
                                                 mybir.ActivationFunctionType.Copy)
                            nc.sync.dma_start(out=out[r * P:(r + 1) * P, :],
                                              in_=ob[:, :])
    return nc


def unpermute_out(prob: Problem, k, raw):
    """raw [LSH, 2D] block-slot-major -> [lsh_true, 2D] real literal order."""
    nblk2, perm = prob.LSH // BW, prob.cores[k].perm2
    real = np.empty_like(raw)
    rr = raw.reshape(nblk2, BW, -1)
    real.reshape(nblk2, BW, -1)[perm] = rr
    return real[:prob.lsh_true]


def fix_library_reload(nc):
    """Encode the (otherwise empty) instr bytes of PseudoReloadLibraryIndex
    so walrus codegen accepts it."""
    isa = nc.isa
    for f in nc.m.functions:
        for b in f.blocks:
            for ins in b.instructions:
                if type(ins).__name__ == "InstPseudoReloadLibraryIndex" or \
                   getattr(ins, "op_name", "") == "PseudoReloadLibraryIndex":
                    instr, fixups = bass_isa.isa_struct(
                        isa, 223,
                        {"pseudo_opcode": 0x2, "lib_index": ins.lib_index},
                        struct_name="NEURON_ISA_TPB_PSEUDO_LIBRARY_RELOAD_INDEX_STRUCT")
                    assert not fixups
                    ins.instr = instr
    return nc


def split_multiwait(nc, max_waits=1, verbose=False):
    import concourse.mybir as mb
    n_fix = 0
    for f in nc.m.functions:
        for b in f.blocks:
            new_insts = []
            for ins in b.instructions:
                si = getattr(ins, "sync_info", None)
                waits = list(si.on_wait) if (si and si.on_wait) else []
                if len(waits) > max_waits:
                    keep = waits[:max_waits]
                    extra = waits[max_waits:]
                    for i, w in enumerate(extra):
                        ev = mb.InstEventSemaphore(
                            name=f"{ins.name}-wsplit{i}",
                            engine=ins.engine,
                            ins=[],
                            outs=[],
                            sync_info=mb.SyncInfo(on_wait=[w], on_update=[]),
                        )
                        new_insts.append(ev)
                        try:
                            nc.register_instruction(ev)
                        except Exception:
                            nc.inst_map[ev.name] = ev
                    ins.sync_info = mb.SyncInfo(
                        on_wait=keep, on_update=list(si.on_update or [])
                    )
                    n_fix += 1
                new_insts.append(ins)
            b.instructions = new_insts
    if verbose:
        print(f"split_multiwait: fixed {n_fix} instructions")
    return nc


# ======================================================================
# harness entry point
# ======================================================================

def kernel(**inputs):
    """Full inputs in, full output out. Shards internally across 8 cores."""
    from concourse.bass_utils import run_bass_kernel_spmd

    NCORES = 8
    L, C, E = 100000, 200000, 800000
    prob, have_bias = prepare(inputs, L, C, E, VLAB=8, D=128, ITERS=3,
                              ncores=NCORES)
    nc = build(prob, have_bias=have_bias)
    split_multiwait(nc)
    fix_library_reload(nc)
    res = run_bass_kernel_spmd(
        nc, [prob.cores[k].in_map for k in range(NCORES)],
        core_ids=list(range(NCORES)))
    out = np.concatenate(
        [unpermute_out(prob, k, res.results[k]["out"]) for k in range(NCORES)],
        axis=0).astype(np.float32)
    return out


# revision 24
# speedup vs baseline: 1.8316x; 1.8316x over previous
"""CNF GNN message-passing kernel for trn2 (8 NeuronCores, SPMD).

Sharding: edges partitioned by DESTINATION (clause shard for l2c, literal
shard for c2l) so each core's local segment-sum is the full sum - no
all-reduce of node features. The linear-layer outputs (Wh tables) are
all-gathered (fp16, split in halves for overlap) so every core can gather
rows for its edges.

Aggregation per direction processes dest BLOCKS of 256 rows (one [P, 256]
psum per block). Edge chunks (128 edges) are pure in (block, source
segment); gathers use InstDMAGatherAnt with int16 in-segment indices,
batched up to 8 chunks (1024 rows) per instruction to amortize the ~1us
SWDGE descriptor-generation overhead (the old per-chunk indirect DMA was
Pool-engine-bound at ~5.3 ms). Blocks are processed in quads so one
gather instruction can span 4 blocks' chunks of the same segment.

Per iteration:
  stage A: Wh_l2c shard = tied_lembs @ Wl  (PE)          -> AllGather #1
  stage C: per block: gather Wh_l2c rows, one-hot (rel==iota)*rc on DVE,
           psum[f, c] += g.T @ oh; relu -> cembs_T [P, 256]
  stage D (fused): Wh_c2l = cembs @ Wc + clause_feat x wcf -> AllGather #2
  stage E: same aggregation into lembs_T; pair-swap for literal tying;
           next stage A matmuls (or final transpose to output).
"""
import numpy as np
from dataclasses import dataclass, field

import concourse.bass as bass
import concourse.bass_isa as bass_isa
import concourse.mybir as mybir
import concourse.tile as tile
from concourse import library_config

F32 = mybir.dt.float32
F16 = mybir.dt.float16
I32 = mybir.dt.int32
I16 = mybir.dt.int16
P = 128
BW = 256            # dest block width (2 ranges of 128)
QUAD = 2            # blocks per processing group
GB = 8              # chunks per dma_gather instruction (8*128 = 1024 idx)
SEG1 = 25088        # whl table segment rows (4 segs over 8*12544)
SEG2 = 28672        # whc table segment rows (7 segs over 8*25088)


# ---------------------------------------------------------------- host prep

@dataclass
class DirData:
    """Per-direction schedule (shared across cores) + per-core arrays."""
    nblk: int
    nseg: int
    NC: int                      # total chunk columns
    sched: list                  # [quad][seg] -> list of (c0, nch)
    chunk_slot: list             # column -> block slot
    slot_nchunks: list           # slot -> total chunks (0 => memset)
    # per-core packed arrays are stored in CoreData.in_map


@dataclass
class CoreData:
    in_map: dict
    perm1: np.ndarray = None
    perm2: np.ndarray = None


@dataclass
class Problem:
    L: int
    C: int
    E: int
    VLAB: int
    D: int
    ITERS: int
    ncores: int
    lsh_true: int
    csh_true: int
    LSH: int
    CSH: int
    NR1: int
    NR2: int
    d1: DirData = None
    d2: DirData = None
    cores: list = field(default_factory=list)


def _pack_idx16(vals_by_col):
    """vals_by_col: [NC][128] int array -> replicated [128, 8*NC] int16."""
    NC = len(vals_by_col)
    arr = np.zeros((16, 8 * NC), np.int16)
    for c, vals in enumerate(vals_by_col):
        v = np.asarray(vals, np.int16).reshape(8, 16)   # i -> [i//16, i%16]
        arr[:, c * 8:(c + 1) * 8] = v.T
    return np.ascontiguousarray(np.tile(arr, (8, 1)))


def _bucket_dir(dst_loc, src_grow, rc_edge, nblk, nseg, segrows):
    """Bucket a core's edges by (block slot, src segment). Returns
    dict[(blk, seg)] -> (idx_in_seg, rel, rc) arrays and per-blk counts."""
    blk = dst_loc // BW
    rel = (dst_loc % BW).astype(np.float32)
    seg = src_grow // segrows
    idxv = (src_grow - seg * segrows).astype(np.int32)
    buckets = {}
    order = np.lexsort((dst_loc, seg, blk))
    blk_s, seg_s = blk[order], seg[order]
    idx_s, rel_s, rc_s = idxv[order], rel[order], rc_edge[order]
    b_edges = np.bincount(blk, minlength=nblk)
    # split on (blk, seg) boundaries
    key = blk_s * nseg + seg_s
    cut = np.flatnonzero(np.diff(key)) + 1
    starts = np.concatenate([[0], cut])
    ends = np.concatenate([cut, [len(key)]])
    for a, b in zip(starts, ends):
        if a == b:
            continue
        buckets[(int(blk_s[a]), int(seg_s[a]))] = (
            idx_s[a:b], rel_s[a:b], rc_s[a:b])
    return buckets, b_edges


def _build_direction(per_core_buckets, per_core_perm, nblk, nseg):
    """Cross-core aligned chunk schedule + per-core packed arrays.

    per_core_buckets[k]: dict[(real_blk, seg)] -> (idx, rel, rc)
    per_core_perm[k]: slot -> real block
    """
    ncores = len(per_core_buckets)
    common = np.zeros((nblk, nseg), np.int64)
    for k in range(ncores):
        perm = per_core_perm[k]
        for (rb, m), (gi, _, _) in per_core_buckets[k].items():
            s = int(np.flatnonzero(perm == rb)[0])
            common[s, m] = max(common[s, m], (len(gi) + P - 1) // P)
    # column layout: quads, then segments, then slots-in-quad
    sched, chunk_slot = [], []
    col = 0
    nquads = (nblk + QUAD - 1) // QUAD
    for q in range(nquads):
        slots_q = list(range(q * QUAD, min((q + 1) * QUAD, nblk)))
        per_m = []
        for m in range(nseg):
            cols_m = []
            for s in slots_q:
                cols_m.extend([s] * int(common[s, m]))
            batches = []
            i0 = 0
            while i0 < len(cols_m):
                nch = min(GB, len(cols_m) - i0)
                batches.append((col + i0, nch))
                i0 += nch
            chunk_slot.extend(cols_m)
            col += len(cols_m)
            per_m.append(batches)
        sched.append(per_m)
    NC = col
    slot_nchunks = [int(common[s].sum()) for s in range(nblk)]

    # per-core packed arrays in the common layout
    packed = []
    for k in range(ncores):
        perm = per_core_perm[k]
        idx_cols, rel_cols, rc_cols = [], [], []
        for q in range(nquads):
            slots_q = list(range(q * QUAD, min((q + 1) * QUAD, nblk)))
            for m in range(nseg):
                for s in slots_q:
                    nch = int(common[s, m])
                    if nch == 0:
                        continue
                    gi, rl, rcv = per_core_buckets[k].get(
                        (int(perm[s]), m), (np.zeros(0, np.int32),
                                            np.zeros(0, np.float32),
                                            np.zeros(0, np.float32)))
                    pad = nch * P - len(gi)
                    gi = np.concatenate([gi, np.zeros(pad, np.int32)])
                    rl = np.concatenate([rl, np.full(pad, -1.0, np.float32)])
                    rcv = np.concatenate([rcv, np.zeros(pad, np.float32)])
                    for j in range(nch):
                        idx_cols.append(gi[j * P:(j + 1) * P])
                        rel_cols.append(rl[j * P:(j + 1) * P])
                        rc_cols.append(rcv[j * P:(j + 1) * P])
        assert len(idx_cols) == NC
        idx16 = _pack_idx16(idx_cols)
        rel = np.stack(rel_cols, axis=1).astype(np.float32)
        rc = np.stack(rc_cols, axis=1).astype(np.float32)
        packed.append((idx16, np.ascontiguousarray(rel),
                       np.ascontiguousarray(rc)))
    dd = DirData(nblk=nblk, nseg=nseg, NC=NC, sched=sched,
                 chunk_slot=chunk_slot, slot_nchunks=slot_nchunks)
    return dd, packed


def prepare(inputs, L, C, E, VLAB=8, D=128, ITERS=3, ncores=8):
    lit_feat = np.asarray(inputs["lit_feat"], np.float32)
    clause_feat = np.asarray(inputs["clause_feat"], np.float32).reshape(-1)
    e_lit = np.asarray(inputs["edge_lit"], np.int64)
    e_cls = np.asarray(inputs["edge_clause"], np.int64)

    assert L % ncores == 0 and C % ncores == 0
    lsh_true, csh_true = L // ncores, C // ncores
    LSH = ((lsh_true + P - 1) // P) * P
    CSH = ((csh_true + P - 1) // P) * P
    NR1, NR2 = CSH // P, LSH // P
    nblk1, nblk2 = CSH // BW, LSH // BW
    LHALF, CHALF = LSH // 2, CSH // 2
    nseg1 = (ncores * LSH + SEG1 - 1) // SEG1
    nseg2 = (ncores * CSH + SEG2 - 1) // SEG2
    assert SEG1 * nseg1 >= ncores * LSH and SEG2 * nseg2 >= ncores * CSH

    cnt_c = np.bincount(e_cls, minlength=C).astype(np.float32)
    cnt_l = np.bincount(e_lit, minlength=L).astype(np.float32)
    rc_c = (1.0 / np.maximum(cnt_c, 1.0)).astype(np.float32)
    rc_l = (1.0 / np.maximum(cnt_l, 1.0)).astype(np.float32)

    W0 = np.asarray(inputs["W_l2c0"], np.float32)
    b0 = np.asarray(inputs["b_l2c0"], np.float32)
    Wl = np.asarray(inputs["W_l2c"], np.float32)
    bl = np.asarray(inputs["b_l2c"], np.float32)
    Wc = np.asarray(inputs["W_c2l"], np.float32)
    bc = np.asarray(inputs["b_c2l"], np.float32)
    have_bias = bool(np.any(b0) or np.any(bl) or np.any(bc))
    Wc_main = np.concatenate([Wc[i, :D, :] for i in range(ITERS)], axis=0)
    wcf = np.stack([Wc[i, D, :] for i in range(ITERS)], axis=0)
    Wl_pack = Wl.reshape((ITERS - 1) * 2 * D, D) if ITERS > 1 else np.zeros((1, D), np.float32)
    bias_pack = np.concatenate(
        [b0[None, :], bl if ITERS > 1 else np.zeros((0, D), np.float32), bc],
        axis=0)

    # per-core block permutations (sorted by block size desc)
    perms1, perms2, inv1, inv2 = [], [], [], []
    sel1s, sel2s = [], []
    for k in range(ncores):
        sel1 = (e_cls >= k * csh_true) & (e_cls < (k + 1) * csh_true)
        loc1 = e_cls[sel1] - k * csh_true
        sizes1 = np.bincount(loc1 // BW, minlength=nblk1)
        p1 = np.argsort(-sizes1, kind="stable")
        perms1.append(p1)
        inv1.append(np.argsort(p1))
        sel1s.append(sel1)
        sel2 = (e_lit >= k * lsh_true) & (e_lit < (k + 1) * lsh_true)
        loc2 = e_lit[sel2] - k * lsh_true
        sizes2 = np.bincount(loc2 // BW, minlength=nblk2)
        p2 = np.argsort(-sizes2, kind="stable")
        perms2.append(p2)
        inv2.append(np.argsort(p2))
        sel2s.append(sel2)

    inv1_all = np.stack(inv1)   # [ncores, nblk1]
    inv2_all = np.stack(inv2)

    def glit_slot(l):
        k, loc = l // lsh_true, l % lsh_true
        Bl, w = loc // BW, loc % BW
        S = inv2_all[k, Bl]
        sr = S * BW + w
        hi = (sr >= LHALF).astype(np.int64)
        return hi * (ncores * LHALF) + k * LHALF + (sr - hi * LHALF)

    def gcls_slot(c):
        k, loc = c // csh_true, c % csh_true
        Bl, w = loc // BW, loc % BW
        S = inv1_all[k, Bl]
        sr = S * BW + w
        hi = (sr >= CHALF).astype(np.int64)
        return hi * (ncores * CHALF) + k * CHALF + (sr - hi * CHALF)

    # bucket edges per core per direction (dest rows in SLOT space)
    bks1, bks2 = [], []
    for k in range(ncores):
        sel1, sel2 = sel1s[k], sel2s[k]
        loc1 = e_cls[sel1] - k * csh_true
        slot_loc1 = inv1_all[k, loc1 // BW] * BW + loc1 % BW
        b1, _ = _bucket_dir(slot_loc1, glit_slot(e_lit[sel1]),
                            rc_c[e_cls[sel1]], nblk1, nseg1, SEG1)
        bks1.append(b1)
        loc2 = e_lit[sel2] - k * lsh_true
        slot_loc2 = inv2_all[k, loc2 // BW] * BW + loc2 % BW
        b2, _ = _bucket_dir(slot_loc2, gcls_slot(e_cls[sel2]),
                            rc_l[e_lit[sel2]], nblk2, nseg2, SEG2)
        bks2.append(b2)

    # schedules reference SLOT ids; buckets keyed by (real?, seg).
    # _build_direction maps slot->real via perm; but buckets above are
    # already keyed by slot-projected... keep keys as (slot, seg):
    # _bucket_dir bucketed by slot_loc//BW == slot directly, so pass
    # identity perms.
    ident1 = [np.arange(nblk1) for _ in range(ncores)]
    ident2 = [np.arange(nblk2) for _ in range(ncores)]
    d1, packed1 = _build_direction(bks1, ident1, nblk1, nseg1)
    d2, packed2 = _build_direction(bks2, ident2, nblk2, nseg2)

    cores = []
    for k in range(ncores):
        idx16_1, rel1, rc1 = packed1[k]
        idx16_2, rel2, rc2 = packed2[k]
        litT0 = np.zeros((VLAB, LSH), np.float32)
        litT0[:, :lsh_true] = lit_feat[k * lsh_true:(k + 1) * lsh_true].T
        litT = np.ascontiguousarray(
            litT0.reshape(VLAB, nblk2, BW)[:, perms2[k], :].reshape(VLAB, LSH))
        cfk_flat = np.zeros(CSH, np.float32)
        cfk_flat[:csh_true] = clause_feat[k * csh_true:(k + 1) * csh_true]
        cfk = np.ascontiguousarray(
            cfk_flat.reshape(nblk1, BW)[perms1[k]].reshape(NR1, P).T)
        iota256 = np.tile(np.arange(BW, dtype=np.float16), (P, 1))
        ident = np.eye(P, dtype=np.float32)
        in_map = dict(
            litT=litT, cf=cfk,
            idx1=idx16_1, rel1=rel1, rc1=rc1,
            idx2=idx16_2, rel2=rel2, rc2=rc2,
            W0=W0, Wc_main=Wc_main, wcf=wcf, Wl_pack=Wl_pack,
            bias_pack=bias_pack, iota256=iota256, ident=ident,
        )
        cores.append(CoreData(in_map, perms1[k], perms2[k]))

    p = Problem(L, C, E, VLAB, D, ITERS, ncores, lsh_true, csh_true,
                LSH, CSH, NR1, NR2, d1, d2, cores)
    return p, have_bias


# ---------------------------------------------------------------- kernel

def build(prob: Problem, have_bias=False, gather_bufs=6,
          oneh_bufs=6, work_bufs=4, psA_bufs=2, psD_bufs=2, psT_bufs=2,
          table_dtype=F16, dma_scratch=16384, nqueues=4):
    d1, d2 = prob.d1, prob.d2
    NC1, NC2 = d1.NC, d2.NC
    L, C, D, VLAB, ITERS = prob.L, prob.C, prob.D, prob.VLAB, prob.ITERS
    LSH, CSH, NR1, NR2 = prob.LSH, prob.CSH, prob.NR1, prob.NR2
    ncores = prob.ncores
    Lfull, Cfull = ncores * LSH, ncores * CSH
    TD = table_dtype

    nc = bass.Bass(dynamic_dma_scratch_size=dma_scratch,
                   num_swdge_queues=nqueues)
    litT = nc.dram_tensor("litT", [VLAB, LSH], F32, kind="ExternalInput")
    cf = nc.dram_tensor("cf", [P, NR1], F32, kind="ExternalInput")
    idx1 = nc.dram_tensor("idx1", [P, 8 * NC1], I16, kind="ExternalInput")
    rel1 = nc.dram_tensor("rel1", [P, NC1], F32, kind="ExternalInput")
    rc1 = nc.dram_tensor("rc1", [P, NC1], F32, kind="ExternalInput")
    idx2 = nc.dram_tensor("idx2", [P, 8 * NC2], I16, kind="ExternalInput")
    rel2 = nc.dram_tensor("rel2", [P, NC2], F32, kind="ExternalInput")
    rc2 = nc.dram_tensor("rc2", [P, NC2], F32, kind="ExternalInput")
    W0 = nc.dram_tensor("W0", [VLAB, D], F32, kind="ExternalInput")
    Wc_main = nc.dram_tensor("Wc_main", [ITERS * D, D], F32, kind="ExternalInput")
    wcf = nc.dram_tensor("wcf", [ITERS, D], F32, kind="ExternalInput")
    Wl_pack = nc.dram_tensor("Wl_pack", [max((ITERS - 1) * 2 * D, 1), D], F32, kind="ExternalInput")
    bias_pack = nc.dram_tensor("bias_pack", [2 * ITERS, D], F32, kind="ExternalInput")
    iota256 = nc.dram_tensor("iota256", [P, BW], F16, kind="ExternalInput")
    identin = nc.dram_tensor("ident", [P, P], F32, kind="ExternalInput")
    out = nc.dram_tensor("out", [LSH, 2 * D], F32, kind="ExternalOutput")

    shared = "Shared" if ncores > 4 else "Local"
    whl_bounce_a = nc.dram_tensor("whl_bounce_a", [LSH // 2, D], TD)
    whl_bounce_b = nc.dram_tensor("whl_bounce_b", [LSH // 2, D], TD)
    whl_full = nc.dram_tensor("whl_full", [Lfull, D], TD, addr_space=shared)
    whc_bounce_a = nc.dram_tensor("whc_bounce_a", [CSH // 2, D], TD)
    whc_bounce_b = nc.dram_tensor("whc_bounce_b", [CSH // 2, D], TD)
    whc_full = nc.dram_tensor("whc_full", [Cfull, D], TD, addr_space=shared)

    rg = [list(range(ncores))]
    LHALF, CHALF = LSH // 2, CSH // 2

    with tile.TileContext(nc) as tc:
        with (
            tc.tile_pool(name="const", bufs=1) as constp,
            tc.tile_pool(name="meta", bufs=1) as metap,
            tc.tile_pool(name="gath", bufs=gather_bufs) as gathp,
            tc.tile_pool(name="oneh", bufs=oneh_bufs) as onehp,
            tc.tile_pool(name="work", bufs=work_bufs) as workp,
            tc.tile_pool(name="outw", bufs=4) as outwp,
            tc.tile_pool(name="lit", bufs=3) as litp,
            tc.tile_pool(name="psA", bufs=psA_bufs, space="PSUM") as psA,
            tc.tile_pool(name="psD", bufs=psD_bufs, space="PSUM") as psD,
            tc.tile_pool(name="psT", bufs=psT_bufs, space="PSUM") as psT,
        ):
            # load the Q7 library for InstDMAGatherAnt before any gather
            nc.gpsimd.load_library(library_config.mlp)

            # one Pool register per distinct num_idxs value (to_reg leaks a
            # fresh register per call otherwise)
            nidx_regs = {}

            def nidx_reg(v):
                if v not in nidx_regs:
                    nidx_regs[v] = nc.gpsimd.to_reg(v)
                return nidx_regs[v]

            # ---- constants
            iota_f = constp.tile([P, BW], F16)
            nc.sync.dma_start(out=iota_f[:], in_=iota256[:, :])
            ident = constp.tile([P, P], F32)
            nc.sync.dma_start(out=ident[:], in_=identin[:, :])

            W0_sb = constp.tile([VLAB, D], F32)
            nc.sync.dma_start(out=W0_sb[:], in_=W0[:, :])
            Wc_sb = constp.tile([P, ITERS * D], F32)
            for i in range(ITERS):
                nc.sync.dma_start(out=Wc_sb[:, i * D:(i + 1) * D],
                                  in_=Wc_main[i * P:(i + 1) * P, :])
            wcf_sb = constp.tile([1, ITERS * D], F32)
            for i in range(ITERS):
                nc.sync.dma_start(out=wcf_sb[:, i * D:(i + 1) * D], in_=wcf[i:i + 1, :])
            if ITERS > 1:
                Wl_sb = constp.tile([P, (ITERS - 1) * 2 * D], F32)
                for i in range(2 * (ITERS - 1)):
                    nc.sync.dma_start(out=Wl_sb[:, i * D:(i + 1) * D],
                                      in_=Wl_pack[i * P:(i + 1) * P, :])
            bias_sb = constp.tile([1, 2 * ITERS * D], F32)
            for i in range(2 * ITERS):
                nc.sync.dma_start(out=bias_sb[:, i * D:(i + 1) * D],
                                  in_=bias_pack[i:i + 1, :])
            ones_sb = constp.tile([1, P], F32)
            nc.vector.memset(ones_sb[:], 1.0)
            cf_sb = constp.tile([P, NR1], F32)
            nc.sync.dma_start(out=cf_sb[:], in_=cf[:, :])

            # ---- edge metadata
            idx1_sb = metap.tile([P, 8 * NC1], I16)
            rel1_sb = metap.tile([P, NC1], F32)
            rc1_sb = metap.tile([P, NC1], F32)
            idx2_sb = metap.tile([P, 8 * NC2], I16)
            rel2_sb = metap.tile([P, NC2], F32)
            rc2_sb = metap.tile([P, NC2], F32)
            for dst, src in [(idx1_sb, idx1), (rel1_sb, rel1), (rc1_sb, rc1),
                             (idx2_sb, idx2), (rel2_sb, rel2), (rc2_sb, rc2)]:
                nc.sync.dma_start(out=dst[:], in_=src[:, :])

            def agg_direction(dd: DirData, idx_sb, rel_sb, rc_sb, table, segrows):
                """Yield (slot, seg256 tile [P, BW]) per dest block after
                aggregation + relu, in slot order."""
                nquads = (dd.nblk + QUAD - 1) // QUAD
                emitted = [0] * dd.nblk       # chunks consumed per slot
                batch_no = [0]
                for q in range(nquads):
                    slots_q = list(range(q * QUAD, min((q + 1) * QUAD, dd.nblk)))
                    ps_of = {}
                    for s in slots_q:
                        if dd.slot_nchunks[s] > 0:
                            ps_agg = psA.tile([P, BW], F32, space="PSUM",
                                              tag=f"agg{s % QUAD}")
                            ps_of[s] = ps_agg[:]
                    for m in range(dd.nseg):
                        for (c0, nch) in dd.sched[q][m]:
                            g = gathp.tile([P, GB, D], TD, tag="g")
                            nc.gpsimd.dma_gather(
                                g[:, 0:nch, :],
                                table[m * segrows:(m + 1) * segrows, :],
                                idx_sb[:, c0 * 8:(c0 + nch) * 8],
                                nch * P, nidx_reg(nch * P), D,
                                queue_num=batch_no[0] % nqueues,
                            )
                            batch_no[0] += 1
                            for j in range(nch):
                                col = c0 + j
                                s = dd.chunk_slot[col]
                                oh = onehp.tile([P, BW], F16, tag="oh")
                                nc.vector.tensor_scalar(
                                    out=oh[:], in0=iota_f[:],
                                    scalar1=rel_sb[:, col:col + 1],
                                    scalar2=rc_sb[:, col:col + 1],
                                    op0=mybir.AluOpType.is_equal,
                                    op1=mybir.AluOpType.mult,
                                )
                                nc.tensor.matmul(
                                    out=ps_of[s], lhsT=g[:, j, :], rhs=oh[:],
                                    start=(emitted[s] == 0),
                                    stop=(emitted[s] == dd.slot_nchunks[s] - 1))
                                emitted[s] += 1
                    for s in slots_q:
                        seg = workp.tile([P, BW], F32, tag="seg")
                        if dd.slot_nchunks[s] == 0:
                            nc.vector.memset(seg[:], 0.0)
                        else:
                            nc.vector.tensor_scalar_max(seg[:], ps_of[s], 0.0)
                        yield s, seg

            def emit_ag(bounces, full, sh, half):
                nc.gpsimd.collective_compute(
                    "AllGather", mybir.AluOpType.bypass,
                    ins=[bounces[half][:].opt()],
                    outs=[full[half * ncores * sh:(half + 1) * ncores * sh, :].opt()],
                    replica_groups=rg)

            def bounce_write(bounces, sh, r, tile_ap):
                half, off = (0, r * P) if r * P < sh else (1, r * P - sh)
                nc.sync.dma_start(out=bounces[half][off:off + P, :], in_=tile_ap)

            whl_bounces = (whl_bounce_a, whl_bounce_b)
            whc_bounces = (whc_bounce_a, whc_bounce_b)

            for it in range(ITERS):
                # ---------- stage A -> whl_bounce (first iteration only)
                if it == 0:
                    for r in range(NR2):
                        lt = litp.tile([VLAB, P], F32, tag="lt")
                        nc.sync.dma_start(out=lt[:], in_=litT[:, r * P:(r + 1) * P])
                        ps = psD.tile([P, D], F32, space="PSUM", tag="ps")
                        nc.tensor.matmul(out=ps[:], lhsT=lt[:], rhs=W0_sb[:],
                                         start=True, stop=not have_bias)
                        if have_bias:
                            nc.tensor.matmul(out=ps[:], lhsT=ones_sb[:],
                                             rhs=bias_sb[:, 0:D],
                                             start=False, stop=True)
                        ot = outwp.tile([P, D], TD, tag="whl_o")
                        nc.scalar.activation(ot[:], ps[:],
                                             mybir.ActivationFunctionType.Copy)
                        bounce_write(whl_bounces, LHALF, r, ot[:])
                        if r == NR2 // 2 - 1:
                            emit_ag(whl_bounces, whl_full, LHALF, 0)
                    emit_ag(whl_bounces, whl_full, LHALF, 1)

                # broadcast wcf[it] across partitions once per iteration
                ps_b = psD.tile([P, D], F32, space="PSUM", tag="ps")
                nc.tensor.matmul(out=ps_b[:], lhsT=ones_sb[:],
                                 rhs=wcf_sb[:, it * D:(it + 1) * D],
                                 start=True, stop=True)
                wcf_bc = workp.tile([P, D], F32, tag="wcfbc")
                nc.scalar.activation(wcf_bc[:], ps_b[:],
                                     mybir.ActivationFunctionType.Copy)

                # ---------- stage C + D: clause blocks
                for s, seg in agg_direction(d1, idx1_sb, rel1_sb, rc1_sb,
                                            whl_full, SEG1):
                    for h in range(2):
                        r = 2 * s + h
                        ps2 = psD.tile([P, D], F32, space="PSUM", tag="ps")
                        nc.tensor.matmul(out=ps2[:],
                                         lhsT=seg[:, h * P:(h + 1) * P],
                                         rhs=Wc_sb[:, it * D:(it + 1) * D],
                                         start=True, stop=not have_bias)
                        if have_bias:
                            nc.tensor.matmul(out=ps2[:], lhsT=ones_sb[:],
                                             rhs=bias_sb[:, (ITERS + it) * D:(ITERS + it + 1) * D],
                                             start=False, stop=True)
                        ot = outwp.tile([P, D], TD, tag="whc_o")
                        nc.vector.scalar_tensor_tensor(
                            out=ot[:], in0=wcf_bc[:], scalar=cf_sb[:, r:r + 1],
                            in1=ps2[:], op0=mybir.AluOpType.mult,
                            op1=mybir.AluOpType.add,
                        )
                        bounce_write(whc_bounces, CHALF, r, ot[:])
                        if r == NR1 // 2 - 1:
                            emit_ag(whc_bounces, whc_full, CHALF, 0)
                emit_ag(whc_bounces, whc_full, CHALF, 1)

                # ---------- stage E: literal blocks
                for s, seg in agg_direction(d2, idx2_sb, rel2_sb, rc2_sb,
                                            whc_full, SEG2):
                    swp = workp.tile([P, BW], F32, tag="swp")
                    nc.vector.tensor_copy(swp[:, 0::2], seg[:, 1::2])
                    nc.vector.tensor_copy(swp[:, 1::2], seg[:, 0::2])
                    for h in range(2):
                        r = 2 * s + h
                        if it < ITERS - 1:
                            ps3 = psD.tile([P, D], F32, space="PSUM", tag="ps")
                            nc.tensor.matmul(out=ps3[:],
                                             lhsT=seg[:, h * P:(h + 1) * P],
                                             rhs=Wl_sb[:, (2 * it) * D:(2 * it + 1) * D],
                                             start=True, stop=False)
                            nc.tensor.matmul(out=ps3[:],
                                             lhsT=swp[:, h * P:(h + 1) * P],
                                             rhs=Wl_sb[:, (2 * it + 1) * D:(2 * it + 2) * D],
                                             start=False, stop=not have_bias)
                            if have_bias:
                                nc.tensor.matmul(out=ps3[:], lhsT=ones_sb[:],
                                                 rhs=bias_sb[:, (1 + it) * D:(2 + it) * D],
                                                 start=False, stop=True)
                            ot = outwp.tile([P, D], TD, tag="whl_o")
                            nc.scalar.activation(ot[:], ps3[:],
                                                 mybir.ActivationFunctionType.Copy)
                            bounce_write(whl_bounces, LHALF, r, ot[:])
                            if r == NR2 // 2 - 1:
                                emit_ag(whl_bounces, whl_full, LHALF, 0)
                            if r == NR2 - 1:
                                emit_ag(whl_bounces, whl_full, LHALF, 1)
                        else:
                            pst = psT.tile([P, P], F32, space="PSUM", tag="tr")
                            nc.tensor.transpose(out=pst[:],
                                                in_=seg[:, h * P:(h + 1) * P],
                                                identity=ident[:])
                            ob = outwp.tile([P, 2 * D], F32, tag="fin")
                            nc.scalar.activation(ob[:, :D], pst[:],
                                                 mybir.ActivationFunctionType.Copy)
                            pst2 = psT.tile([P, P], F32, space="PSUM", tag="tr")
                            nc.tensor.transpose(out=pst2[:],
                                                in_=swp[:, h * P:(h + 1) * P],
                                                identity=ident[:])
                            nc.scalar.activation(ob[:, D:], pst2[:],
                                                 mybir.ActivationFunctionType.Copy)
                            nc.sync.dma_start(out=out[r * P:(r + 1) * P, :],
                                              in_=ob[:, :])
    return nc


def unpermute_out(prob: Problem, k, raw):
    """raw [LSH, 2D] block-slot-major -> [lsh_true, 2D] real literal order."""
    nblk2, perm = prob.LSH // BW, prob.cores[k].perm2
    real = np.empty_like(raw)
    rr = raw.reshape(nblk2, BW, -1)
    real.reshape(nblk2, BW, -1)[perm] = rr
    return real[:prob.lsh_true]


def fix_library_reload(nc):
    """Encode the (otherwise empty) instr bytes of PseudoReloadLibraryIndex
    so walrus codegen accepts it."""
    isa = nc.isa
    for f in nc.m.functions:
        for b in f.blocks:
            for ins in b.instructions:
                if type(ins).__name__ == "InstPseudoReloadLibraryIndex" or \
                   getattr(ins, "op_name", "") == "PseudoReloadLibraryIndex":
                    instr, fixups = bass_isa.isa_struct(
                        isa, 223,
                        {"pseudo_opcode": 0x2, "lib_index": ins.lib_index},
                        struct_name="NEURON_ISA_TPB_PSEUDO_LIBRARY_RELOAD_INDEX_STRUCT")
                    assert not fixups
                    ins.instr = instr
    return nc


def split_multiwait(nc, max_waits=1, verbose=False):
    import concourse.mybir as mb
    n_fix = 0
    for f in nc.m.functions:
        for b in f.blocks:
            new_insts = []
            for ins in b.instructions:
                si = getattr(ins, "sync_info", None)
                waits = list(si.on_wait) if (si and si.on_wait) else []
                if len(waits) > max_waits:
                    keep = waits[:max_waits]
                    extra = waits[max_waits:]
                    for i, w in enumerate(extra):
                        ev = mb.InstEventSemaphore(
                            name=f"{ins.name}-wsplit{i}",
                            engine=ins.engine,
                            ins=[],
                            outs=[],
                            sync_info=mb.SyncInfo(on_wait=[w], on_update=[]),
                        )
                        new_insts.append(ev)
                        try:
                            nc.register_instruction(ev)
                        except Exception:
                            nc.inst_map[ev.name] = ev
                    ins.sync_info = mb.SyncInfo(
                        on_wait=keep, on_update=list(si.on_update or [])
                    )
                    n_fix += 1
                new_insts.append(ins)
            b.instructions = new_insts
    if verbose:
        print(f"split_multiwait: fixed {n_fix} instructions")
    return nc


# ======================================================================
# harness entry point
# ======================================================================

def kernel(**inputs):
    """Full inputs in, full output out. Shards internally across 8 cores."""
    from concourse.bass_utils import run_bass_kernel_spmd

    NCORES = 8
    L, C, E = 100000, 200000, 800000
    prob, have_bias = prepare(inputs, L, C, E, VLAB=8, D=128, ITERS=3,
                              ncores=NCORES)
    nc = build(prob, have_bias=have_bias)
    split_multiwait(nc)
    fix_library_reload(nc)
    res = run_bass_kernel_spmd(
        nc, [prob.cores[k].in_map for k in range(NCORES)],
        core_ids=list(range(NCORES)))
    out = np.concatenate(
        [unpermute_out(prob, k, res.results[k]["out"]) for k in range(NCORES)],
        axis=0).astype(np.float32)
    return out
